# revision 1
# baseline (speedup 1.0000x reference)
"""Trainium2 Bass kernel for nn_CNNModel (gnn_message_passing).

Model: 3 sparse grouped 3x3 convs (fan-in 3/16/32, SAME pad, relu) on
[64,32,32,3] -> flatten -> dense(131072,50)+relu -> dense(50,10) -> softmax.

Sharding (8 cores): spatial over H. Core k computes L2 output rows
[4k, 4k+4) for ALL 64 images (halo rows recomputed per core: L1 rows
[4k-1,4k+5), L0 rows [4k-2,4k+6)), then the dense1 partial product over
its 16384 flattened features, a tiny [50,64] AllReduce, and a replicated
dense2+softmax tail. Convs run as channels-in-partitions shift-and-matmul
in float32r (full PE rate at N>=256):
  - L0 as one K=28 matmul per chunk (host-packed im2col, 27 tap-channel
    rows + 1 boundary-poison row whose -1e9 weight zeroes out-of-image
    rows through the relu).
  - L1 with x-tap pairing (block0/block1 shifted copies in partitions)
    -> 6 matmuls per chunk instead of 9.
  - L2 dense 128->128 per tap (the mod-4 sparsity cannot reduce PE
    cycles without 4x activation copies).
Out-of-image L1 rows are zeroed with a per-core If(partition_id) memset.
"""
import os
import sys

if "/opt/trn_rl_repo" not in sys.path:
    sys.path.insert(0, "/opt/trn_rl_repo")

# Persistent XLA compilation cache: the NEFF (neuronxcc, 10+ min) is built
# inside the jit compile, so caching the executable skips recompiles across
# processes for an unchanged kernel.
os.environ.setdefault("JAX_COMPILATION_CACHE_DIR", "/tmp/jax_comp_cache")
os.environ.setdefault("JAX_PERSISTENT_CACHE_MIN_COMPILE_TIME_SECS", "1")
os.environ.setdefault("JAX_PERSISTENT_CACHE_MIN_ENTRY_SIZE_BYTES", "0")

from contextlib import ExitStack

import numpy as np

NCORES = 8
B, H, W = 64, 32, 32
NEG = -1.0e9

_CACHE = {}


def _make_idx():
    I = np.eye(16)
    w1 = sum(np.roll(I, shift=j, axis=0) for j in range(4))
    w2 = sum(np.roll(I, shift=4 * j, axis=0) for j in range(4))
    conn1 = np.kron(np.ones((8, 4)), w1)  # [128, 64]
    conn2 = np.kron(np.ones((8, 8)), w2)  # [128, 128]
    idx1 = np.stack([np.nonzero(conn1[r])[0] for r in range(128)])
    idx2 = np.stack([np.nonzero(conn2[r])[0] for r in range(128)])
    return idx1, idx2


def _dense_w(cw, idx, cin):
    # cw [3,3,F,n], idx [n,F] -> dense [3,3,cin,n]
    n, _ = idx.shape
    wd = np.zeros((3, 3, cin, n), np.float32)
    for node in range(n):
        wd[:, :, idx[node], node] = cw[:, :, :, node]
    return wd


def _build_p0(inputs, k):
    """Host im2col pack for core k: [28, B*8*32].

    Row (ky*3+kx)*3+c at (b, r, x) = input[b, (4k-2+r)+ky-1, x+kx-1, c]
    (zero-padded). Row 27 = 1.0 on out-of-image L0 rows (poison indicator).
    """
    p = np.zeros((28, B, 8, 32), np.float32)
    xpad = np.zeros((B, H + 8, W + 2, 3), np.float32)
    xpad[:, 4 : 4 + H, 1 : 1 + W, :] = inputs
    for ky in range(3):
        for kx in range(3):
            for c in range(3):
                row = (ky * 3 + kx) * 3 + c
                g0 = 4 * k - 2 + ky - 1 + 4  # padded row index for r=0
                p[row] = xpad[:, g0 : g0 + 8, kx : kx + 32, c]
    for r in range(8):
        g = 4 * k - 2 + r
        if g < 0 or g >= H:
            p[27, :, r, :] = 1.0
    return np.ascontiguousarray(p.reshape(28, -1))


def _build_nc():
    import concourse.tile as tile
    from concourse import bacc, mybir

    FP = mybir.dt.float32
    FPR = mybir.dt.float32r
    FH = mybir.dt.float16
    AF = mybir.ActivationFunctionType
    AX = mybir.AxisListType

    nc = bacc.Bacc("TRN2", target_bir_lowering=False, debug=False, num_devices=NCORES)

    p0_d = nc.dram_tensor("p0", [28, 16384], FPR, kind="ExternalInput")
    w0_d = nc.dram_tensor("w0", [28, 64], FPR, kind="ExternalInput")
    w1p_d = nc.dram_tensor("w1p", [3, 128, 128], FH, kind="ExternalInput")
    w1s_d = nc.dram_tensor("w1s", [3, 64, 128], FH, kind="ExternalInput")
    w2t_d = nc.dram_tensor("w2t", [9, 128, 128], FH, kind="ExternalInput")
    b0_d = nc.dram_tensor("b0", [64, 1], FP, kind="ExternalInput")
    b1_d = nc.dram_tensor("b1", [128, 1], FP, kind="ExternalInput")
    b2_d = nc.dram_tensor("b2", [128, 1], FP, kind="ExternalInput")
    # host-pretransposed to [c, i*50+j] so the load is contiguous per partition
    dw1_d = nc.dram_tensor("dw1k", [128, 6400], FP, kind="ExternalInput")
    db1_d = nc.dram_tensor("db1", [50, 1], FP, kind="ExternalInput")
    dw2_d = nc.dram_tensor("dw2a", [51, 10], FP, kind="ExternalInput")
    ones_d = nc.dram_tensor("onesrow", [1, 64], FP, kind="ExternalInput")
    out_d = nc.dram_tensor("out", [64, 10], FP, kind="ExternalOutput")

    with tile.TileContext(nc) as tc, ExitStack() as top:
        consts = top.enter_context(tc.tile_pool(name="consts", bufs=1))
        acts = top.enter_context(tc.tile_pool(name="acts", bufs=1))
        drams = top.enter_context(tc.tile_pool(name="drams", bufs=1, space="DRAM"))
        psum0 = top.enter_context(tc.tile_pool(name="psum0", bufs=2, space="PSUM"))
        psum1 = top.enter_context(tc.tile_pool(name="psum1", bufs=2, space="PSUM"))
        psum2 = top.enter_context(tc.tile_pool(name="psum2", bufs=2, space="PSUM"))
        psumd = top.enter_context(tc.tile_pool(name="psumd", bufs=1, space="PSUM"))
        psume = top.enter_context(tc.tile_pool(name="psume", bufs=1, space="PSUM"))

        # ---- constant loads (issued first so DMA prefetch overlaps) ----
        w0s = consts.tile([28, 64], FPR)
        nc.sync.dma_start(w0s[:], w0_d[:, :])
        w1ps = []
        for ky in range(3):
            t = consts.tile([128, 128], FH, name=f"w1p{ky}")
            nc.sync.dma_start(t[:], w1p_d[ky, :, :])
            w1ps.append(t)
        w1ss = []
        for ky in range(3):
            t = consts.tile([64, 128], FH, name=f"w1s{ky}")
            nc.sync.dma_start(t[:], w1s_d[ky, :, :])
            w1ss.append(t)
        w2ts = []
        for tt in range(9):
            t = consts.tile([128, 128], FH, name=f"w2t{tt}")
            nc.sync.dma_start(t[:], w2t_d[tt, :, :])
            w2ts.append(t)
        b0s = consts.tile([64, 1], FP)
        nc.sync.dma_start(b0s[:], b0_d[:, :])
        b1s = consts.tile([128, 1], FP)
        nc.sync.dma_start(b1s[:], b1_d[:, :])
        b2s = consts.tile([128, 1], FP)
        nc.sync.dma_start(b2s[:], b2_d[:, :])
        db1s = consts.tile([50, 1], FP)
        nc.sync.dma_start(db1s[:], db1_d[:, :])
        dw2s = consts.tile([51, 10], FP)
        nc.sync.dma_start(dw2s[:], dw2_d[:, :])

        # ---- persistent activation buffers ----
        l0out = acts.tile([128, 64 * 8 * 34], FH)  # [2x64ch, b, r(8), w(34)]
        l0v = l0out.rearrange("p (b r w) -> p b r w", b=64, r=8, w=34)
        l1out = acts.tile([128, 64 * 6 * 34], FH)  # [128ch, b, s(6), w(34)]
        l1v = l1out.rearrange("p (b s w) -> p b s w", b=64, s=6, w=34)

        # zero the w-pad columns (x=-1 / x=32)
        nc.vector.memset(l0v[0:64, :, :, 0:1], 0.0)
        nc.vector.memset(l0v[0:64, :, :, 33:34], 0.0)
        nc.vector.memset(l1v[:, :, :, 0:1], 0.0)
        nc.vector.memset(l1v[:, :, :, 33:34], 0.0)

        # ---- L0: K=28 single-matmul conv (im2col packed on host) ----
        with tc.tile_pool(name="p0pool", bufs=2) as p0pool:
            for g in range(8):
                p0t = p0pool.tile([28, 2048], FPR, tag="p0t")
                nc.sync.dma_start(p0t[:], p0_d[:, 2048 * g : 2048 * (g + 1)])
                for u in range(4):
                    i = g * 4 + u  # image pair index
                    ps = psum0.tile([64, 512], FP, tag="ps0")
                    nc.tensor.matmul(
                        ps[:],
                        w0s[:],
                        p0t[:, 512 * u : 512 * (u + 1)],
                        start=True,
                        stop=True,
                    )
                    psv = ps.rearrange("p (b r w) -> p b r w", b=2, r=8, w=32)
                    nc.scalar.activation(
                        l0v[0:64, 2 * i : 2 * i + 2, :, 1:33],
                        psv[:, :, :, :],
                        AF.Relu,
                        bias=b0s[:, 0:1],
                    )
                    # block1 = block0 shifted one x to the left (tap pairing)
                    nc.vector.tensor_copy(
                        l0v[64:128, 2 * i : 2 * i + 2, :, 0:33],
                        l0v[0:64, 2 * i : 2 * i + 2, :, 1:34],
                    )

        # ---- L1: 3 paired (K=128) + 3 single (K=64) matmuls per chunk ----
        for j in range(32):
            ps = psum1.tile([128, 384], FP, tag="ps1")
            for ky in range(3):
                nc.tensor.matmul(
                    ps[:],
                    w1ps[ky][:],
                    l0v[0:128, 2 * j : 2 * j + 2, ky : ky + 6, 0:32],
                    start=(ky == 0),
                    stop=False,
                )
            for ky in range(3):
                nc.tensor.matmul(
                    ps[:],
                    w1ss[ky][:],
                    l0v[0:64, 2 * j : 2 * j + 2, ky : ky + 6, 2:34],
                    start=False,
                    stop=(ky == 2),
                )
            psv = ps.rearrange("p (b s w) -> p b s w", b=2, s=6, w=32)
            nc.scalar.activation(
                l1v[:, 2 * j : 2 * j + 2, :, 1:33],
                psv[:, :, :, :],
                AF.Relu,
                bias=b1s[:, 0:1],
            )

        # zero out-of-image L1 rows (core 0: global row -1; core 7: row 32)
        pid = nc.partition_id()
        with tc.If(pid == 0):
            nc.vector.memset(l1v[:, :, 0:1, :], 0.0)
        with tc.If(pid == 7):
            nc.vector.memset(l1v[:, :, 5:6, :], 0.0)

        # ---- L2 (dense 128->128 per tap) + dense1 ----
        with tc.tile_pool(name="acts2", bufs=1) as acts2:
            l2out = acts2.tile([128, 64 * 4 * 32], FP)  # [128ch, b, t(4), x(32)]
            l2v = l2out.rearrange("p (b t x) -> p b t x", b=64, t=4, x=32)
            dw1s = acts2.tile([128, 128 * 50], FP)
            dw1v = dw1s.rearrange("p (i j) -> p i j", i=128, j=50)
            nc.gpsimd.dma_start(dw1s[:], dw1_d[:, :])

            for q in range(16):
                ps = psum2.tile([128, 512], FP, tag="ps2")
                t = 0
                for ky in range(3):
                    for kx in range(3):
                        nc.tensor.matmul(
                            ps[:],
                            w2ts[t][:],
                            l1v[:, 4 * q : 4 * q + 4, ky : ky + 4, kx : kx + 32],
                            start=(t == 0),
                            stop=(t == 8),
                        )
                        t += 1
                psv = ps.rearrange("p (b t x) -> p b t x", b=4, t=4, x=32)
                nc.scalar.activation(
                    l2v[:, 4 * q : 4 * q + 4, :, :],
                    psv[:, :, :, :],
                    AF.Relu,
                    bias=b2s[:, 0:1],
                )

            # dense1 partial: accumulate 128 K-tiles into psum [50, 64]
            psd = psumd.tile([50, 64], FP)
            for i in range(128):
                nc.tensor.matmul(
                    psd[:],
                    dw1v[:, i, :],
                    l2v[:, :, i // 32, i % 32],
                    start=(i == 0),
                    stop=(i == 127),
                )

            # ---- AllReduce the [50, 64] partial across the 8 cores ----
            ar_s = consts.tile([50, 64], FP)
            nc.vector.tensor_copy(ar_s[:], psd[:])
            in_b = drams.tile([50, 64], FP)
            out_b = drams.tile([50, 64], FP)
            nc.sync.dma_start(in_b[:], ar_s[:])
            nc.gpsimd.collective_compute(
                "AllReduce",
                mybir.AluOpType.add,
                replica_groups=[list(range(NCORES))],
                ins=[in_b.opt()],
                outs=[out_b.opt()],
            )
            ar_o = consts.tile([50, 64], FP)
            nc.sync.dma_start(ar_o[:], out_b[:])

            # ---- dense2 + softmax (replicated tail) ----
            y1 = consts.tile([51, 64], FP)
            nc.sync.dma_start(y1[50:51, :], ones_d[:, :])
            nc.scalar.activation(y1[0:50, :], ar_o[:], AF.Relu, bias=db1s[:, 0:1])
            pse = psume.tile([64, 10], FP)
            nc.tensor.matmul(pse[:], y1[:], dw2s[:], start=True, stop=True)
            mx = consts.tile([64, 1], FP)
            nc.vector.reduce_max(mx[:], pse[:], axis=AX.X)
            mxn = consts.tile([64, 1], FP)
            nc.scalar.mul(mxn[:], mx[:], -1.0)
            ex = consts.tile([64, 10], FP)
            nc.scalar.activation(ex[:], pse[:], AF.Exp, bias=mxn[:, 0:1])
            sm = consts.tile([64, 1], FP)
            nc.vector.reduce_sum(sm[:], ex[:], axis=AX.X)
            rc = consts.tile([64, 1], FP)
            nc.vector.reciprocal(rc[:], sm[:])
            outs = consts.tile([64, 10], FP)
            nc.vector.tensor_scalar_mul(outs[:], ex[:], rc[:, 0:1])
            nc.sync.dma_start(out_d[:, :], outs[:])

    nc.finalize()
    return nc


def _get_nc():
    if "nc" not in _CACHE:
        _CACHE["nc"] = _build_nc()
    return _CACHE["nc"]


def _prep_in_maps(inputs, cw0, cb0, cw1, cb1, cw2, cb2, dw1, db1, dw2, db2):
    idx1, idx2 = _make_idx()
    w1d = _dense_w(np.asarray(cw1, np.float32), idx1, 64)
    w2d = _dense_w(np.asarray(cw2, np.float32), idx2, 128)

    w0 = np.concatenate(
        [np.asarray(cw0, np.float32).reshape(27, 64), np.full((1, 64), NEG, np.float32)],
        axis=0,
    )
    w1p = np.ascontiguousarray(
        np.stack([np.concatenate([w1d[ky, 0], w1d[ky, 1]], axis=0) for ky in range(3)])
    ).astype(np.float16)
    w1s = np.ascontiguousarray(np.stack([w1d[ky, 2] for ky in range(3)])).astype(np.float16)
    w2t = np.ascontiguousarray(w2d.reshape(9, 128, 128)).astype(np.float16)
    dw2a = np.ascontiguousarray(
        np.concatenate([np.asarray(dw2, np.float32), np.asarray(db2, np.float32)[None, :]], axis=0)
    )
    common = {
        "w0": np.ascontiguousarray(w0),
        "w1p": w1p,
        "w1s": w1s,
        "w2t": w2t,
        "b0": np.ascontiguousarray(np.asarray(cb0, np.float32)[:, None]),
        "b1": np.ascontiguousarray(np.asarray(cb1, np.float32)[:, None]),
        "b2": np.ascontiguousarray(np.asarray(cb2, np.float32)[:, None]),
        "db1": np.ascontiguousarray(np.asarray(db1, np.float32)[:, None]),
        "dw2a": dw2a,
        "onesrow": np.ones((1, 64), np.float32),
    }
    x = np.asarray(inputs, np.float32).reshape(B, H, W, 3)
    dw1 = np.asarray(dw1, np.float32)
    in_maps = []
    for k in range(NCORES):
        m = dict(common)
        m["p0"] = _build_p0(x, k)
        # [16384, 50] -> [i, c, j] -> [c, i*50+j]
        m["dw1k"] = np.ascontiguousarray(
            dw1[k * 16384 : (k + 1) * 16384].reshape(128, 128, 50).transpose(1, 0, 2).reshape(128, 6400)
        )
        in_maps.append(m)
    return in_maps


def _run(inputs_dict, trace=False):
    from concourse.bass_utils import run_bass_kernel_spmd

    nc = _get_nc()
    in_maps = _prep_in_maps(**inputs_dict)
    res = run_bass_kernel_spmd(
        nc, in_maps, core_ids=list(range(NCORES)), trace=trace
    )
    out = np.asarray(res.results[0]["out"], np.float32)
    return out, res


def kernel(**inputs):
    out, _ = _run(inputs, trace=False)
    return out

